# revision 59
# baseline (speedup 1.0000x reference)
"""Trainium2 Bass kernel for NextDiT joint self+cross attention block.

Sharding: TP=4 over heads x DP=2 over batch (8 cores).
Each core handles one batch (dp = core // 4) and 4 heads (tp = core % 4).

Dataflow (per core, everything in "transposed" [feature, token] layout):
  x -> (PE transpose) -> x^T (bf16)
  Q^T = wq_loc^T x  (columns host-permuted: per head, even dims then odd dims,
                     so RoPE pairs are contiguous partition halves)
  K^T likewise; V natural via stationary x^T tiles.
  LN stats (sum, sumsq over local j) via ones-matmuls -> packed AllReduce
  over the 4-core TP group -> replicated [128, S] r/m tiles -> LN apply + RoPE.
  Per head: scores^T[kk, q] = Kn^T(stationary) . Qn^T -> exp (ACT, fp32)
  -> denominator tile-adds (DVE) + ones-matmul; PV: out^T += V(stationary) . expS^T.
  Cross-attn identical with y-derived K/V (no RoPE), gated by tanh(gate).
  Final: partial = out^T(stationary) . wo_loc (f32r) -> ReduceScatter(add) over
  the TP group -> each core returns its 512-token shard; host concatenates.
"""

import math
from dataclasses import dataclass

import ml_dtypes
import numpy as np

import concourse.bass as bass
import concourse.mybir as mybir
import concourse.tile as tile
from concourse import bacc, bass_utils
from concourse.masks import make_identity

FP32 = mybir.dt.float32
F32R = mybir.dt.float32r
BF16 = mybir.dt.bfloat16

LN_EPS = 1e-5


@dataclass(frozen=True)
class Cfg:
    S: int = 2048   # x tokens per batch
    D: int = 2048   # model dim (= H * HD)
    H: int = 16     # total heads
    HD: int = 128   # head dim
    YL: int = 256   # y tokens per batch
    YD: int = 2048  # y model dim
    TP: int = 4     # tensor-parallel ways (heads)
    DP: int = 2     # data-parallel ways (batch)

    @property
    def HL(self):
        return self.H // self.TP

    @property
    def JL(self):
        return self.HL * self.HD


def emit(nc, cfg: Cfg):
    """Declare DRAM I/O and emit the Tile kernel."""
    S, D, HD, YL, YD = cfg.S, cfg.D, cfg.HD, cfg.YL, cfg.YD
    HL, JL, TP, DP = cfg.HL, cfg.JL, cfg.TP, cfg.DP
    NT = S // 128    # x token tiles
    NI = D // 128    # model-dim chunks
    NJ = JL // 128   # local head tiles (= HL since HD == 128)
    NTY = YL // 128  # y token tiles
    NIY = YD // 128  # y model-dim chunks
    QC = min(1024, S)        # q chunk for attention score tiles (2 psum banks)
    NQC = S // QC
    MMQ = min(512, QC)       # matmul N within a q chunk
    SGQ = QC // MMQ
    assert HD == 128

    statlen = 4 * S + 2 * YL

    # ---- DRAM I/O ----
    x_d = nc.dram_tensor("x", [S, D], FP32, kind="ExternalInput")
    y_d = nc.dram_tensor("y", [YL, YD], FP32, kind="ExternalInput")
    # c2 = [cos; cos], s2 = [-sin; sin] stacked over the permuted head dim
    cos_d = nc.dram_tensor("cos2", [HD, S], BF16, kind="ExternalInput")
    sin_d = nc.dram_tensor("sin2", [HD, S], BF16, kind="ExternalInput")
    wq_d = nc.dram_tensor("wq", [D, JL], BF16, kind="ExternalInput")
    wk_d = nc.dram_tensor("wk", [D, JL], BF16, kind="ExternalInput")
    wv_d = nc.dram_tensor("wv", [D, JL], BF16, kind="ExternalInput")
    wky_d = nc.dram_tensor("wky", [YD, JL], BF16, kind="ExternalInput")
    wvy_d = nc.dram_tensor("wvy", [YD, JL], BF16, kind="ExternalInput")
    wo_d = nc.dram_tensor("wo", [JL, D], F32R, kind="ExternalInput")
    qnw_d = nc.dram_tensor("qn_w", [HD, HL], FP32, kind="ExternalInput")
    qnb_d = nc.dram_tensor("qn_b", [HD, HL], FP32, kind="ExternalInput")
    knw_d = nc.dram_tensor("kn_w", [HD, HL], FP32, kind="ExternalInput")
    knb_d = nc.dram_tensor("kn_b", [HD, HL], FP32, kind="ExternalInput")
    kynw_d = nc.dram_tensor("kyn_w", [HD, HL], FP32, kind="ExternalInput")
    kynb_d = nc.dram_tensor("kyn_b", [HD, HL], FP32, kind="ExternalInput")
    gate_d = nc.dram_tensor("gate", [1, HL], FP32, kind="ExternalInput")
    out_d = nc.dram_tensor("out_shard", [S // TP, D], BF16, kind="ExternalOutput")

    groups = [[g * TP + i for i in range(TP)] for g in range(DP)]

    with tile.TileContext(nc) as tc, nc.allow_low_precision(
        reason="bf16 intermediates are within the checked tolerance"
    ):
        with (
            tc.tile_pool(name="const", bufs=1) as const,
            tc.tile_pool(name="dram", bufs=1, space="DRAM") as dram,
            tc.tile_pool(name="midp", bufs=1) as midp,
        ):
            # ---------- constants ----------
            ident = const.tile([128, 128], BF16)
            make_identity(nc, ident)
            ones_bf = const.tile([128, 1], BF16)
            nc.gpsimd.memset(ones_bf, 1.0)
            ones_f32sq = const.tile([128, 128], FP32)
            nc.gpsimd.memset(ones_f32sq, 1.0)
            ones1_f32 = const.tile([1, 128], FP32)
            nc.gpsimd.memset(ones1_f32, 1.0)
            eps_t = const.tile([128, 1], FP32)
            nc.gpsimd.memset(eps_t, LN_EPS)
            cos_sb = const.tile([HD, S], BF16)
            nc.sync.dma_start(cos_sb[:], cos_d.ap())
            sin_sb = const.tile([HD, S], BF16)
            nc.sync.dma_start(sin_sb[:], sin_d.ap())
            nrm = {}
            for nm, d in [
                ("qn_w", qnw_d), ("qn_b", qnb_d), ("kn_w", knw_d),
                ("kn_b", knb_d), ("kyn_w", kynw_d), ("kyn_b", kynb_d),
            ]:
                t = const.tile([HD, HL], FP32, name=f"nrm_{nm}")
                nc.sync.dma_start(t[:], d.ap())
                nrm[nm] = t
            gate_raw = const.tile([1, HL], FP32)
            nc.sync.dma_start(gate_raw[:], gate_d.ap())
            gate_t = const.tile([1, HL], FP32)
            nc.scalar.activation(gate_t[:], gate_raw[:],
                                 mybir.ActivationFunctionType.Tanh)
            gate_b = const.tile([128, HL], FP32)
            nc.gpsimd.partition_broadcast(gate_b[:], gate_t[:])

            # bounce buffers
            stats_in = dram.tile([1, statlen], FP32)
            stats_out = dram.tile([1, statlen], FP32)
            rs_in = dram.tile([S, D], BF16)
            rs_out = dram.tile([S // TP, D], BF16)

            # midp (stage1->4): V tiles and Q/K/Ky tiles. The Q/K/Ky tiles are
            # written as raw projections in stage 1, then LN+RoPE'd IN PLACE
            # in stage 2 (qn_sb aliases rawq etc.).
            v_sb = [midp.tile([128, JL], BF16, name=f"v{t}") for t in range(NT)]
            vy_sb = [midp.tile([128, JL], BF16, name=f"vy{t}")
                     for t in range(NTY)]
            rawq = [midp.tile([128, S], BF16, name=f"qh{j}") for j in range(NJ)]
            rawk = [midp.tile([128, S], BF16, name=f"kh{j}") for j in range(NJ)]
            rawky = [midp.tile([128, YL], BF16, name=f"kyh{j}")
                     for j in range(NJ)]
            qn_sb, kn_sb, kyn_sb = rawq, rawk, rawky  # in-place LN+RoPE

            if True:
                # ============ stage 1: transposes + projections =============
                with (
                    tc.tile_pool(name="xT", bufs=1) as xTp,
                    tc.tile_pool(name="wts", bufs=NI + NI // 2) as wts,
                    tc.tile_pool(name="xin", bufs=2) as xinp,
                    tc.tile_pool(name="tp_ps", bufs=4, space="PSUM") as tp_ps,
                    tc.tile_pool(name="proj_ps", bufs=2, space="PSUM") as proj_ps,
                    tc.tile_pool(name="st_ps", bufs=2, space="PSUM") as st_ps,
                    tc.tile_pool(name="sq", bufs=3) as sqp,
                ):
                    xT = [xTp.tile([128, S], BF16, name=f"xT{i}")
                          for i in range(NI)]
                    yT = [xTp.tile([128, YL], BF16, name=f"yT{i}")
                          for i in range(NIY)]

                    def load_w(d, n_in, nm):
                        tl = []
                        for i in range(n_in):
                            t = wts.tile([128, JL], BF16, tag="w",
                                         name=f"{nm}_{i}")
                            nc.sync.dma_start(
                                t[:], d.ap()[i * 128:(i + 1) * 128, :])
                            tl.append(t)
                        return tl

                    # helper: one token-group of a Q^T/K^T-style projection
                    def proj_g(w_t, xTs, n_in, ntok, raws, stat_off, g):
                        nq = min(512, ntok)
                        ssum = st_ps.tile([1, nq], FP32, tag="st")
                        ssq = st_ps.tile([1, nq], FP32, tag="st")
                        for jt in range(NJ):
                            ps = proj_ps.tile([128, nq], FP32, tag="proj")
                            for it in range(n_in):
                                nc.tensor.matmul(
                                    ps[:],
                                    lhsT=w_t[it][:, jt * 128:(jt + 1) * 128],
                                    rhs=xTs[it][:, g * nq:(g + 1) * nq],
                                    start=(it == 0), stop=(it == n_in - 1))
                            nc.any.tensor_copy(
                                raws[jt][:, g * nq:(g + 1) * nq], ps[:])
                            sq = sqp.tile([128, nq], BF16, tag="sq")
                            nc.vector.tensor_mul(
                                sq[:], raws[jt][:, g * nq:(g + 1) * nq],
                                raws[jt][:, g * nq:(g + 1) * nq])
                            nc.tensor.matmul(
                                ssum[:], lhsT=ones_bf[:],
                                rhs=raws[jt][:, g * nq:(g + 1) * nq],
                                start=(jt == 0), stop=(jt == NJ - 1))
                            nc.tensor.matmul(
                                ssq[:], lhsT=ones_bf[:], rhs=sq[:],
                                start=(jt == 0), stop=(jt == NJ - 1))
                        st_stage = sqp.tile([1, 2 * nq], FP32,
                                            tag="st_stage", bufs=1)
                        nc.any.tensor_copy(st_stage[0:1, 0:nq], ssum[:])
                        nc.any.tensor_copy(st_stage[0:1, nq:2 * nq], ssq[:])
                        nc.sync.dma_start(
                            stats_in[0:1,
                                     stat_off + g * nq:stat_off + (g + 1) * nq],
                            st_stage[0:1, 0:nq])
                        nc.sync.dma_start(
                            stats_in[0:1,
                                     stat_off + ntok + g * nq:
                                     stat_off + ntok + (g + 1) * nq],
                            st_stage[0:1, nq:2 * nq])

                    # y^T transposes
                    for tt in range(NTY):
                        xin = xinp.tile([128, D], FP32, tag="xin")
                        nc.sync.dma_start(
                            xin[:], y_d.ap()[tt * 128:(tt + 1) * 128, :])
                        xb = xinp.tile([128, D], BF16, tag="xb")
                        nc.vector.tensor_copy(xb[:], xin[:])
                        for it in range(NIY):
                            ps = tp_ps.tile([128, 128], BF16, tag="tp")
                            nc.tensor.transpose(
                                ps[:], xb[:, it * 128:(it + 1) * 128],
                                ident[:])
                            nc.vector.tensor_copy(
                                yT[it][:, tt * 128:(tt + 1) * 128], ps[:])

                    # x^T transposes (dense), then weight-major projections
                    for tt in range(NT):
                        xin = xinp.tile([128, D], FP32, tag="xin")
                        nc.sync.dma_start(
                            xin[:], x_d.ap()[tt * 128:(tt + 1) * 128, :])
                        xb = xinp.tile([128, D], BF16, tag="xb")
                        nc.vector.tensor_copy(xb[:], xin[:])
                        for it in range(NI):
                            ps = tp_ps.tile([128, 128], BF16, tag="tp")
                            nc.tensor.transpose(
                                ps[:], xb[:, it * 128:(it + 1) * 128],
                                ident[:])
                            nc.vector.tensor_copy(
                                xT[it][:, tt * 128:(tt + 1) * 128], ps[:])
                    wq_t = load_w(wq_d, NI, "wq")
                    for g in range(S // 512):
                        proj_g(wq_t, xT, NI, S, rawq, 0, g)
                    wk_t = load_w(wk_d, NI, "wk")
                    for g in range(S // 512):
                        proj_g(wk_t, xT, NI, S, rawk, 2 * S, g)
                    wky_t = load_w(wky_d, NIY, "wky")
                    proj_g(wky_t, yT, NIY, YL, rawky, 4 * S, 0)

                    # V / Vy (natural layout, stationary = x^T tiles)
                    for w_d2, xTs, n_in, ntok, dsts, nm in (
                        (wv_d, xT, NI, NT, v_sb, "wv"),
                        (wvy_d, yT, NIY, NTY, vy_sb, "wvy"),
                    ):
                        w_t = load_w(w_d2, n_in, nm)
                        for t_i in range(ntok):
                            ps = proj_ps.tile([128, JL], FP32, tag="proj")
                            for it in range(n_in):
                                nc.tensor.matmul(
                                    ps[:],
                                    lhsT=xTs[it][:, t_i * 128:(t_i + 1) * 128],
                                    rhs=w_t[it][:],
                                    start=(it == 0), stop=(it == n_in - 1))
                            nc.any.tensor_copy(dsts[t_i][:], ps[:])

                # out_sb lives stage3->4; its pool opens only now, after the
                # stage-1 pools (xT/weights) released their space.
                outp_cm = tc.tile_pool(name="outp", bufs=1)
                outp = outp_cm.__enter__()
                out_sb = [outp.tile([128, S], F32R, name=f"outh{j}")
                          for j in range(NJ)]

                # ====== stage 2+3: stats AR, then per-head LN/rope+attention =
                # two ARs: q-side stats are ready early, so its AR + LN chain
                # hides under the remaining projections.
                nc.gpsimd.collective_compute(
                    "AllReduce", mybir.AluOpType.add,
                    replica_groups=groups,
                    ins=[stats_in[0:1, 0:2 * S].opt()],
                    outs=[stats_out[0:1, 0:2 * S].opt()],
                )
                nc.gpsimd.collective_compute(
                    "AllReduce", mybir.AluOpType.add,
                    replica_groups=groups,
                    ins=[stats_in[0:1, 2 * S:4 * S].opt()],
                    outs=[stats_out[0:1, 2 * S:4 * S].opt()],
                )
                nc.gpsimd.collective_compute(
                    "AllReduce", mybir.AluOpType.add,
                    replica_groups=groups,
                    ins=[stats_in[0:1, 4 * S:statlen].opt()],
                    outs=[stats_out[0:1, 4 * S:statlen].opt()],
                )

                with (
                    tc.tile_pool(name="stat_sb", bufs=1) as stat_sbp,
                    tc.tile_pool(name="ln_tmp", bufs=2) as lnt,
                    tc.tile_pool(name="rmr", bufs=1) as rmrp,
                    tc.tile_pool(name="pv_ps", bufs=3, space="PSUM") as pv_ps,
                    tc.tile_pool(name="sc_ps", bufs=2, space="PSUM") as sc_ps,
                    tc.tile_pool(name="den_ps2", bufs=1,
                                 space="PSUM") as den_ps2,
                    tc.tile_pool(name="fin_ps", bufs=1, space="PSUM") as fin_ps,
                    tc.tile_pool(name="fin_sb", bufs=2) as fin_sb,
                    tc.tile_pool(name="es", bufs=5) as esp,
                    tc.tile_pool(name="rden", bufs=2) as rdenp,
                ):
                    r_q = rmrp.tile([128, S], BF16)
                    mr_q = rmrp.tile([128, S], BF16)
                    r_k = rmrp.tile([128, S], BF16)
                    mr_k = rmrp.tile([128, S], BF16)
                    r_ky = rmrp.tile([128, YL], BF16)
                    mr_ky = rmrp.tile([128, YL], BF16)

                    st_sb = stat_sbp.tile([1, statlen], FP32)
                    nc.sync.dma_start(st_sb[0:1, 0:2 * S],
                                      stats_out[0:1, 0:2 * S])
                    nc.sync.dma_start(st_sb[0:1, 2 * S:4 * S],
                                      stats_out[0:1, 2 * S:4 * S])
                    nc.sync.dma_start(st_sb[0:1, 4 * S:statlen],
                                      stats_out[0:1, 4 * S:statlen])
                    st_fr = st_sb

                    def make_rm(stat_off, ntok, r_t, mr_t):
                        nq = min(512, ntok)
                        for g in range(ntok // nq):
                            s_sl = st_fr[0:1,
                                         stat_off + g * nq:
                                         stat_off + (g + 1) * nq]
                            q_sl = st_fr[0:1,
                                         stat_off + ntok + g * nq:
                                         stat_off + ntok + (g + 1) * nq]
                            rsum = sc_ps.tile([128, nq], FP32, tag="sc")
                            rsq = sc_ps.tile([128, nq], FP32, tag="sc")
                            nc.tensor.matmul(rsum[:], lhsT=ones1_f32[:],
                                             rhs=s_sl, start=True, stop=True)
                            nc.tensor.matmul(rsq[:], lhsT=ones1_f32[:],
                                             rhs=q_sl, start=True, stop=True)
                            mean = lnt.tile([128, nq], FP32, tag="ln_mean", bufs=1)
                            nc.vector.tensor_scalar_mul(mean[:], rsum[:],
                                                        1.0 / D)
                            t_a = lnt.tile([128, nq], FP32, tag="ln_ta", bufs=1)
                            nc.vector.tensor_mul(t_a[:], mean[:], mean[:])
                            t_b = lnt.tile([128, nq], FP32, tag="ln_tb", bufs=1)
                            nc.vector.tensor_scalar_mul(t_b[:], rsq[:], 1.0 / D)
                            nc.vector.tensor_sub(t_b[:], t_b[:], t_a[:])
                            nc.scalar.activation(
                                t_a[:], t_b[:],
                                mybir.ActivationFunctionType.Sqrt,
                                bias=eps_t[:], scale=1.0)
                            nc.vector.reciprocal(
                                r_t[:, g * nq:(g + 1) * nq], t_a[:])
                            nc.vector.tensor_mul(
                                mr_t[:, g * nq:(g + 1) * nq], mean[:],
                                r_t[:, g * nq:(g + 1) * nq])

                    make_rm(0, S, r_q, mr_q)
                    make_rm(2 * S, S, r_k, mr_k)
                    make_rm(4 * S, YL, r_ky, mr_ky)

                    def ln_rope_head(jt, raws, r_t, mr_t, wnm, bnm, ntok,
                                     do_rope):
                        t1 = lnt.tile([128, ntok], BF16, tag="ln_t1", bufs=1)
                        nc.vector.tensor_mul(t1[:], raws[jt][:], r_t[:, :ntok])
                        nc.vector.tensor_sub(t1[:], t1[:], mr_t[:, :ntok])
                        nc.vector.tensor_scalar(
                            t1[:], t1[:], nrm[wnm][:, jt:jt + 1],
                            nrm[bnm][:, jt:jt + 1],
                            op0=mybir.AluOpType.mult, op1=mybir.AluOpType.add)
                        if not do_rope:
                            nc.vector.tensor_copy(raws[jt][:], t1[:])
                            return
                        hh = HD // 2
                        tsw = lnt.tile([128, ntok], BF16, tag="rope_sw", bufs=1)
                        nc.sync.dma_start(tsw[0:hh, :], t1[hh:HD, :])
                        nc.sync.dma_start(tsw[hh:HD, :], t1[0:hh, :])
                        p1 = lnt.tile([128, ntok], BF16, tag="rope_p1", bufs=1)
                        nc.vector.tensor_mul(p1[:], tsw[:], sin_sb[:, :ntok])
                        nc.vector.tensor_mul(raws[jt][:], t1[:],
                                             cos_sb[:, :ntok])
                        nc.vector.tensor_add(raws[jt][:], raws[jt][:], p1[:])

                    ones_bf128 = stat_sbp.tile([128, 128], BF16)
                    nc.gpsimd.memset(ones_bf128, 1.0)

                    wo_sb = []
                    for jc in range(NJ):
                        t = stat_sbp.tile([128, D], F32R, name=f"wo{jc}")
                        nc.sync.dma_start(
                            t[:], wo_d.ap()[jc * 128:(jc + 1) * 128, :])
                        wo_sb.append(t)

                    QW = 512
                    NQCH = S // QW

                    def attend_q(kns, vs, nkk, ht, q0, dst_op):
                        pv = pv_ps.tile([128, QW], FP32, tag="pv")
                        denr = den_ps2.tile([128, QW], FP32, tag="denr")
                        pend = []

                        def flush_one():
                            if pend:
                                pend.pop(0)()

                        for kkc in range(nkk):
                            sc = sc_ps.tile([128, QW], FP32, tag="sc")
                            nc.tensor.matmul(
                                sc[:], lhsT=kns[kkc],
                                rhs=qn_sb[ht][:, q0:q0 + QW],
                                start=True, stop=True)
                            es = esp.tile([128, QW], BF16, tag="es")
                            nc.scalar.activation(
                                es[:], sc[:],
                                mybir.ActivationFunctionType.Exp)
                            nc.tensor.matmul(
                                denr[:], lhsT=ones_bf128[:], rhs=es[:],
                                start=(kkc == 0), stop=(kkc == nkk - 1))

                            def mk_pv(kkc=kkc, es=es):
                                def run():
                                    nc.tensor.matmul(
                                        pv[:],
                                        lhsT=vs[kkc][:,
                                                     ht * 128:(ht + 1) * 128],
                                        rhs=es[:],
                                        start=(kkc == 0),
                                        stop=(kkc == nkk - 1))
                                return run
                            pend.append(mk_pv())
                            if len(pend) > 3:
                                flush_one()
                        while pend:
                            flush_one()
                        rden = rdenp.tile([128, QW], BF16, tag="rden")
                        nc.vector.reciprocal(rden[:], denr[:])
                        dst_op(pv, rden, q0)

                    NTC = QW // 128          # token tiles per q-chunk
                    NGF = D // 512           # N-groups in final proj
                    pending_fin = []
                    for qch in range(NQCH):
                        q0 = qch * QW
                        for ht in range(NJ):
                            if ht == 1 and pending_fin:
                                pending_fin.pop(0)()
                            if qch == 0:
                                # LN + rope just-in-time: head ht's attention
                                # starts as soon as its own LN is done.
                                ln_rope_head(ht, rawq, r_q, mr_q, "qn_w",
                                             "qn_b", S, True)
                                ln_rope_head(ht, rawk, r_k, mr_k, "kn_w",
                                             "kn_b", S, True)
                                ln_rope_head(ht, rawky, r_ky, mr_ky, "kyn_w",
                                             "kyn_b", YL, False)
                            kn_slices = [kn_sb[ht][:, c * 128:(c + 1) * 128]
                                         for c in range(NT)]

                            def self_out(pv, rden, q0, ht=ht):
                                nc.vector.tensor_mul(
                                    out_sb[ht][:, q0:q0 + QW], pv[:], rden[:])

                            attend_q(kn_slices, v_sb, NT, ht, q0, self_out)

                            kyn_slices = [kyn_sb[ht][:, c * 128:(c + 1) * 128]
                                          for c in range(NTY)]

                            def cross_out(pv, rden, q0, ht=ht):
                                nc.vector.tensor_scalar_mul(
                                    rden[:], rden[:], gate_b[:, ht:ht + 1])
                                tmp = rdenp.tile([128, QW], BF16,
                                                 tag="cross_tmp")
                                nc.vector.tensor_mul(tmp[:], pv[:], rden[:])
                                nc.vector.tensor_add(
                                    out_sb[ht][:, q0:q0 + QW],
                                    out_sb[ht][:, q0:q0 + QW], tmp[:])

                            attend_q(kyn_slices, vy_sb, NTY, ht, q0, cross_out)

                        # final projection + reduce-scatter, deferred
                        # by one q-chunk so attention matmuls hide its deps
                        def emit_fin(qch=qch):
                            NDH = 2 if NGF % 2 == 0 else 1
                            DH = D // NDH
                            for tt in range(qch * NTC, (qch + 1) * NTC):
                                for dh in range(NDH):
                                    fp = fin_ps.tile([128, DH], FP32,
                                                     tag="fin")
                                    NW = min(512, DH)
                                    for jc in range(NJ):
                                        for ng in range(DH // NW):
                                            c0 = dh * DH + ng * NW
                                            nc.tensor.matmul(
                                                fp[:, ng * NW:(ng + 1) * NW],
                                                lhsT=out_sb[jc][
                                                    :,
                                                    tt * 128:(tt + 1) * 128],
                                                rhs=wo_sb[jc][:, c0:c0 + NW],
                                                start=(jc == 0),
                                                stop=(jc == NJ - 1))
                                    fs = fin_sb.tile([128, DH], BF16,
                                                     tag="fstage")
                                    nc.vector.tensor_copy(fs[:], fp[:])
                                    nc.sync.dma_start(
                                        rs_in[tt * 128:(tt + 1) * 128,
                                              dh * DH:(dh + 1) * DH],
                                        fs[:])
                            nc.gpsimd.collective_compute(
                                "ReduceScatter", mybir.AluOpType.add,
                                replica_groups=groups,
                                ins=[rs_in[qch * QW:(qch + 1) * QW,
                                           :].opt()],
                                outs=[rs_out[qch * (QW // TP):
                                             (qch + 1) * (QW // TP),
                                             :].opt()],
                            )
                            nc.sync.dma_start(
                                out_d.ap()[qch * (QW // TP):
                                           (qch + 1) * (QW // TP), :],
                                rs_out[qch * (QW // TP):
                                       (qch + 1) * (QW // TP), :])
                        pending_fin.append(emit_fin)
                    while pending_fin:
                        pending_fin.pop(0)()
            outp_cm.__exit__(None, None, None)


# ======================= host side =========================================

def _perm_cols(cfg: Cfg, tp: int):
    """Global wq/wk/wky column indices for core tp: local heads with
    per-head even/odd interleave -> evens-first permutation."""
    cols = []
    for h in range(tp * cfg.HL, (tp + 1) * cfg.HL):
        base = h * cfg.HD
        cols.extend(range(base, base + cfg.HD, 2))
        cols.extend(range(base + 1, base + cfg.HD, 2))
    return np.asarray(cols)


def _nat_cols(cfg: Cfg, tp: int):
    return np.arange(tp * cfg.JL, (tp + 1) * cfg.JL)


def make_in_maps(cfg: Cfg, inputs: dict):
    bf = ml_dtypes.bfloat16
    sqhd = 1.0 / math.sqrt(cfg.HD)
    cos_t = np.asarray(inputs["freqs_cos"]).T.astype(np.float32)
    sin_t = np.asarray(inputs["freqs_sin"]).T.astype(np.float32)
    c2 = np.ascontiguousarray(np.vstack([cos_t, cos_t])).astype(bf)
    s2 = np.ascontiguousarray(np.vstack([-sin_t, sin_t])).astype(bf)
    inputs = {k: np.asarray(v) for k, v in inputs.items()}
    in_maps = []
    for c in range(cfg.TP * cfg.DP):
        dp, tp = divmod(c, cfg.TP)
        pc = _perm_cols(cfg, tp)
        ncol = _nat_cols(cfg, tp)

        def headcols(v, cols):
            return np.ascontiguousarray(
                v[cols].reshape(cfg.HL, cfg.HD).T).astype(np.float32)

        in_maps.append({
            "x": np.ascontiguousarray(inputs["x"][dp]).astype(np.float32),
            "y": np.ascontiguousarray(inputs["y"][dp]).astype(np.float32),
            "cos2": c2,
            "sin2": s2,
            "wq": np.ascontiguousarray(inputs["wq"][:, pc]).astype(bf),
            "wk": np.ascontiguousarray(inputs["wk"][:, pc]).astype(bf),
            "wv": np.ascontiguousarray(inputs["wv"][:, ncol]).astype(bf),
            "wky": np.ascontiguousarray(inputs["wk_y"][:, pc]).astype(bf),
            "wvy": np.ascontiguousarray(inputs["wv_y"][:, ncol]).astype(bf),
            "wo": np.ascontiguousarray(inputs["wo"][ncol, :]).astype(np.float32),
            # q-side norm params carry the 1/sqrt(HD) attention scale
            "qn_w": headcols(inputs["q_norm_w"] * sqhd, pc),
            "qn_b": headcols(inputs["q_norm_b"] * sqhd, pc),
            "kn_w": headcols(inputs["k_norm_w"], pc),
            "kn_b": headcols(inputs["k_norm_b"], pc),
            "kyn_w": headcols(inputs["ky_norm_w"], pc),
            "kyn_b": headcols(inputs["ky_norm_b"], pc),
            "gate": np.ascontiguousarray(
                inputs["gate"][tp * cfg.HL:(tp + 1) * cfg.HL][None, :]
            ).astype(np.float32),
        })
    return in_maps


def assemble(cfg: Cfg, results):
    B = cfg.DP
    out = np.empty((B, cfg.S, cfg.D), np.float32)
    NCH = cfg.S // 512          # one RS chunk per 512-token q-chunk
    chrows = 512
    shrows = chrows // cfg.TP
    for c in range(cfg.TP * cfg.DP):
        dp, tp = divmod(c, cfg.TP)
        sh = np.asarray(results[c]["out_shard"]).astype(np.float32)
        for ch in range(NCH):
            out[dp, ch * chrows + tp * shrows:
                ch * chrows + (tp + 1) * shrows, :] = \
                sh[ch * shrows:(ch + 1) * shrows]
    return out


_CACHE = {}


def build(cfg: Cfg):
    if cfg in _CACHE:
        return _CACHE[cfg]
    nc = bacc.Bacc("TRN2", target_bir_lowering=False, debug=False,
                   num_devices=cfg.TP * cfg.DP)
    emit(nc, cfg)
    nc.compile()
    _CACHE[cfg] = nc
    return nc


def kernel(**inputs) -> np.ndarray:
    cfg = Cfg()
    nc = build(cfg)
    in_maps = make_in_maps(cfg, inputs)
    res = bass_utils.run_bass_kernel_spmd(
        nc, in_maps, core_ids=list(range(cfg.TP * cfg.DP)))
    return assemble(cfg, res.results)


# revision 60
# speedup vs baseline: 1.1370x; 1.1370x over previous
"""Trainium2 Bass kernel for NextDiT joint self+cross attention block.

Sharding: TP=4 over heads x DP=2 over batch (8 cores).
Each core handles one batch (dp = core // 4) and 4 heads (tp = core % 4).

Dataflow (per core, everything in "transposed" [feature, token] layout):
  x -> (PE transpose) -> x^T (bf16)
  Q^T = wq_loc^T x  (columns host-permuted: per head, even dims then odd dims,
                     so RoPE pairs are contiguous partition halves)
  K^T likewise; V natural via stationary x^T tiles.
  LN stats (sum, sumsq over local j) via ones-matmuls -> packed AllReduce
  over the 4-core TP group -> replicated [128, S] r/m tiles -> LN apply + RoPE.
  Per head: scores^T[kk, q] = Kn^T(stationary) . Qn^T -> exp (ACT, fp32)
  -> denominator tile-adds (DVE) + ones-matmul; PV: out^T += V(stationary) . expS^T.
  Cross-attn identical with y-derived K/V (no RoPE), gated by tanh(gate).
  Final: partial = out^T(stationary) . wo_loc (f32r) -> ReduceScatter(add) over
  the TP group -> each core returns its 512-token shard; host concatenates.
"""

import math
from dataclasses import dataclass

import ml_dtypes
import numpy as np

import concourse.bass as bass
import concourse.mybir as mybir
import concourse.tile as tile
from concourse import bacc, bass_utils
from concourse.masks import make_identity

FP32 = mybir.dt.float32
F32R = mybir.dt.float32r
BF16 = mybir.dt.bfloat16

LN_EPS = 1e-5


@dataclass(frozen=True)
class Cfg:
    S: int = 2048   # x tokens per batch
    D: int = 2048   # model dim (= H * HD)
    H: int = 16     # total heads
    HD: int = 128   # head dim
    YL: int = 256   # y tokens per batch
    YD: int = 2048  # y model dim
    TP: int = 4     # tensor-parallel ways (heads)
    DP: int = 2     # data-parallel ways (batch)

    @property
    def HL(self):
        return self.H // self.TP

    @property
    def JL(self):
        return self.HL * self.HD


def emit(nc, cfg: Cfg):
    """Declare DRAM I/O and emit the Tile kernel."""
    S, D, HD, YL, YD = cfg.S, cfg.D, cfg.HD, cfg.YL, cfg.YD
    HL, JL, TP, DP = cfg.HL, cfg.JL, cfg.TP, cfg.DP
    NT = S // 128    # x token tiles
    NI = D // 128    # model-dim chunks
    NJ = JL // 128   # local head tiles (= HL since HD == 128)
    NTY = YL // 128  # y token tiles
    NIY = YD // 128  # y model-dim chunks
    QC = min(1024, S)        # q chunk for attention score tiles (2 psum banks)
    NQC = S // QC
    MMQ = min(512, QC)       # matmul N within a q chunk
    SGQ = QC // MMQ
    assert HD == 128

    statlen = 4 * S + 2 * YL

    # ---- DRAM I/O ----
    x_d = nc.dram_tensor("x", [S, D], FP32, kind="ExternalInput")
    y_d = nc.dram_tensor("y", [YL, YD], FP32, kind="ExternalInput")
    # c2 = [cos; cos], s2 = [-sin; sin] stacked over the permuted head dim
    cos_d = nc.dram_tensor("cos2", [HD, S], BF16, kind="ExternalInput")
    sin_d = nc.dram_tensor("sin2", [HD, S], BF16, kind="ExternalInput")
    wq_d = nc.dram_tensor("wq", [D, JL], BF16, kind="ExternalInput")
    wk_d = nc.dram_tensor("wk", [D, JL], BF16, kind="ExternalInput")
    wv_d = nc.dram_tensor("wv", [D, JL], BF16, kind="ExternalInput")
    wky_d = nc.dram_tensor("wky", [YD, JL], BF16, kind="ExternalInput")
    wvy_d = nc.dram_tensor("wvy", [YD, JL], BF16, kind="ExternalInput")
    wo_d = nc.dram_tensor("wo", [JL, D], F32R, kind="ExternalInput")
    qnw_d = nc.dram_tensor("qn_w", [HD, HL], FP32, kind="ExternalInput")
    qnb_d = nc.dram_tensor("qn_b", [HD, HL], FP32, kind="ExternalInput")
    knw_d = nc.dram_tensor("kn_w", [HD, HL], FP32, kind="ExternalInput")
    knb_d = nc.dram_tensor("kn_b", [HD, HL], FP32, kind="ExternalInput")
    kynw_d = nc.dram_tensor("kyn_w", [HD, HL], FP32, kind="ExternalInput")
    kynb_d = nc.dram_tensor("kyn_b", [HD, HL], FP32, kind="ExternalInput")
    gate_d = nc.dram_tensor("gate", [1, HL], FP32, kind="ExternalInput")
    out_d = nc.dram_tensor("out_shard", [S // TP, D], BF16, kind="ExternalOutput")

    groups = [[g * TP + i for i in range(TP)] for g in range(DP)]

    with tile.TileContext(nc) as tc, nc.allow_low_precision(
        reason="bf16 intermediates are within the checked tolerance"
    ):
        with (
            tc.tile_pool(name="const", bufs=1) as const,
            tc.tile_pool(name="dram", bufs=1, space="DRAM") as dram,
            tc.tile_pool(name="midp", bufs=1) as midp,
        ):
            # ---------- constants ----------
            ident = const.tile([128, 128], BF16)
            make_identity(nc, ident)
            ones_bf = const.tile([128, 1], BF16)
            nc.gpsimd.memset(ones_bf, 1.0)
            ones_f32sq = const.tile([128, 128], FP32)
            nc.gpsimd.memset(ones_f32sq, 1.0)
            ones1_f32 = const.tile([1, 128], FP32)
            nc.gpsimd.memset(ones1_f32, 1.0)
            eps_t = const.tile([128, 1], FP32)
            nc.gpsimd.memset(eps_t, LN_EPS)
            cos_sb = const.tile([HD, S], BF16)
            nc.sync.dma_start(cos_sb[:], cos_d.ap())
            sin_sb = const.tile([HD, S], BF16)
            nc.sync.dma_start(sin_sb[:], sin_d.ap())
            nrm = {}
            for nm, d in [
                ("qn_w", qnw_d), ("qn_b", qnb_d), ("kn_w", knw_d),
                ("kn_b", knb_d), ("kyn_w", kynw_d), ("kyn_b", kynb_d),
            ]:
                t = const.tile([HD, HL], FP32, name=f"nrm_{nm}")
                nc.sync.dma_start(t[:], d.ap())
                nrm[nm] = t
            gate_raw = const.tile([1, HL], FP32)
            nc.sync.dma_start(gate_raw[:], gate_d.ap())
            gate_t = const.tile([1, HL], FP32)
            nc.scalar.activation(gate_t[:], gate_raw[:],
                                 mybir.ActivationFunctionType.Tanh)
            gate_b = const.tile([128, HL], FP32)
            nc.gpsimd.partition_broadcast(gate_b[:], gate_t[:])

            # bounce buffers
            stats_in = dram.tile([1, statlen], FP32)
            stats_out = dram.tile([1, statlen], FP32)
            rs_in = dram.tile([S, D], BF16)
            rs_out = dram.tile([S // TP, D], BF16)

            # midp (stage1->4): V tiles and Q/K/Ky tiles. The Q/K/Ky tiles are
            # written as raw projections in stage 1, then LN+RoPE'd IN PLACE
            # in stage 2 (qn_sb aliases rawq etc.).
            v_sb = [midp.tile([128, JL], BF16, name=f"v{t}") for t in range(NT)]
            vy_sb = [midp.tile([128, JL], BF16, name=f"vy{t}")
                     for t in range(NTY)]
            rawq = [midp.tile([128, S], BF16, name=f"qh{j}") for j in range(NJ)]
            rawk = [midp.tile([128, S], BF16, name=f"kh{j}") for j in range(NJ)]
            rawky = [midp.tile([128, YL], BF16, name=f"kyh{j}")
                     for j in range(NJ)]
            qn_sb, kn_sb, kyn_sb = rawq, rawk, rawky  # in-place LN+RoPE

            if True:
                # ============ stage 1: transposes + projections =============
                with (
                    tc.tile_pool(name="xT", bufs=1) as xTp,
                    tc.tile_pool(name="wts", bufs=NI + NI // 2) as wts,
                    tc.tile_pool(name="xin", bufs=2) as xinp,
                    tc.tile_pool(name="tp_ps", bufs=4, space="PSUM") as tp_ps,
                    tc.tile_pool(name="proj_ps", bufs=2, space="PSUM") as proj_ps,
                    tc.tile_pool(name="st_ps", bufs=2, space="PSUM") as st_ps,
                    tc.tile_pool(name="sq", bufs=3) as sqp,
                ):
                    xT = [xTp.tile([128, S], BF16, name=f"xT{i}")
                          for i in range(NI)]
                    yT = [xTp.tile([128, YL], BF16, name=f"yT{i}")
                          for i in range(NIY)]

                    def load_w(d, n_in, nm):
                        tl = []
                        for i in range(n_in):
                            t = wts.tile([128, JL], BF16, tag="w",
                                         name=f"{nm}_{i}")
                            nc.sync.dma_start(
                                t[:], d.ap()[i * 128:(i + 1) * 128, :])
                            tl.append(t)
                        return tl

                    # helper: one token-group of a Q^T/K^T-style projection
                    def proj_g(w_t, xTs, n_in, ntok, raws, stat_off, g):
                        nq = min(512, ntok)
                        ssum = st_ps.tile([1, nq], FP32, tag="st")
                        ssq = st_ps.tile([1, nq], FP32, tag="st")
                        for jt in range(NJ):
                            ps = proj_ps.tile([128, nq], FP32, tag="proj")
                            for it in range(n_in):
                                nc.tensor.matmul(
                                    ps[:],
                                    lhsT=w_t[it][:, jt * 128:(jt + 1) * 128],
                                    rhs=xTs[it][:, g * nq:(g + 1) * nq],
                                    start=(it == 0), stop=(it == n_in - 1))
                            nc.any.tensor_copy(
                                raws[jt][:, g * nq:(g + 1) * nq], ps[:])
                            sq = sqp.tile([128, nq], BF16, tag="sq")
                            nc.vector.tensor_mul(
                                sq[:], raws[jt][:, g * nq:(g + 1) * nq],
                                raws[jt][:, g * nq:(g + 1) * nq])
                            nc.tensor.matmul(
                                ssum[:], lhsT=ones_bf[:],
                                rhs=raws[jt][:, g * nq:(g + 1) * nq],
                                start=(jt == 0), stop=(jt == NJ - 1))
                            nc.tensor.matmul(
                                ssq[:], lhsT=ones_bf[:], rhs=sq[:],
                                start=(jt == 0), stop=(jt == NJ - 1))
                        st_stage = sqp.tile([1, 2 * nq], FP32,
                                            tag="st_stage", bufs=1)
                        nc.any.tensor_copy(st_stage[0:1, 0:nq], ssum[:])
                        nc.any.tensor_copy(st_stage[0:1, nq:2 * nq], ssq[:])
                        nc.sync.dma_start(
                            stats_in[0:1,
                                     stat_off + g * nq:stat_off + (g + 1) * nq],
                            st_stage[0:1, 0:nq])
                        nc.sync.dma_start(
                            stats_in[0:1,
                                     stat_off + ntok + g * nq:
                                     stat_off + ntok + (g + 1) * nq],
                            st_stage[0:1, nq:2 * nq])

                    # y^T transposes
                    for tt in range(NTY):
                        xin = xinp.tile([128, D], FP32, tag="xin")
                        nc.sync.dma_start(
                            xin[:], y_d.ap()[tt * 128:(tt + 1) * 128, :])
                        xb = xinp.tile([128, D], BF16, tag="xb")
                        nc.vector.tensor_copy(xb[:], xin[:])
                        for it in range(NIY):
                            ps = tp_ps.tile([128, 128], BF16, tag="tp")
                            nc.tensor.transpose(
                                ps[:], xb[:, it * 128:(it + 1) * 128],
                                ident[:])
                            nc.vector.tensor_copy(
                                yT[it][:, tt * 128:(tt + 1) * 128], ps[:])

                    # x^T transposes (dense), then weight-major projections
                    for tt in range(NT):
                        xin = xinp.tile([128, D], FP32, tag="xin")
                        nc.sync.dma_start(
                            xin[:], x_d.ap()[tt * 128:(tt + 1) * 128, :])
                        xb = xinp.tile([128, D], BF16, tag="xb")
                        nc.vector.tensor_copy(xb[:], xin[:])
                        for it in range(NI):
                            ps = tp_ps.tile([128, 128], BF16, tag="tp")
                            nc.tensor.transpose(
                                ps[:], xb[:, it * 128:(it + 1) * 128],
                                ident[:])
                            nc.vector.tensor_copy(
                                xT[it][:, tt * 128:(tt + 1) * 128], ps[:])
                    wq_t = load_w(wq_d, NI, "wq")
                    for g in range(S // 512):
                        proj_g(wq_t, xT, NI, S, rawq, 0, g)
                    wk_t = load_w(wk_d, NI, "wk")
                    for g in range(S // 512):
                        proj_g(wk_t, xT, NI, S, rawk, 2 * S, g)
                    wky_t = load_w(wky_d, NIY, "wky")
                    proj_g(wky_t, yT, NIY, YL, rawky, 4 * S, 0)

                    # V / Vy (natural layout, stationary = x^T tiles)
                    for w_d2, xTs, n_in, ntok, dsts, nm in (
                        (wv_d, xT, NI, NT, v_sb, "wv"),
                        (wvy_d, yT, NIY, NTY, vy_sb, "wvy"),
                    ):
                        w_t = load_w(w_d2, n_in, nm)
                        for t_i in range(ntok):
                            ps = proj_ps.tile([128, JL], FP32, tag="proj")
                            for it in range(n_in):
                                nc.tensor.matmul(
                                    ps[:],
                                    lhsT=xTs[it][:, t_i * 128:(t_i + 1) * 128],
                                    rhs=w_t[it][:],
                                    start=(it == 0), stop=(it == n_in - 1))
                            nc.any.tensor_copy(dsts[t_i][:], ps[:])

                # out_sb lives stage3->4; its pool opens only now, after the
                # stage-1 pools (xT/weights) released their space.
                outp_cm = tc.tile_pool(name="outp", bufs=1)
                outp = outp_cm.__enter__()
                out_sb = [outp.tile([128, S], F32R, name=f"outh{j}")
                          for j in range(NJ)]

                # ====== stage 2+3: stats AR, then per-head LN/rope+attention =
                # two ARs: q-side stats are ready early, so its AR + LN chain
                # hides under the remaining projections.
                nc.gpsimd.collective_compute(
                    "AllReduce", mybir.AluOpType.add,
                    replica_groups=groups,
                    ins=[stats_in[0:1, 0:2 * S].opt()],
                    outs=[stats_out[0:1, 0:2 * S].opt()],
                )
                nc.gpsimd.collective_compute(
                    "AllReduce", mybir.AluOpType.add,
                    replica_groups=groups,
                    ins=[stats_in[0:1, 2 * S:4 * S].opt()],
                    outs=[stats_out[0:1, 2 * S:4 * S].opt()],
                )
                nc.gpsimd.collective_compute(
                    "AllReduce", mybir.AluOpType.add,
                    replica_groups=groups,
                    ins=[stats_in[0:1, 4 * S:statlen].opt()],
                    outs=[stats_out[0:1, 4 * S:statlen].opt()],
                )

                with (
                    tc.tile_pool(name="stat_sb", bufs=1) as stat_sbp,
                    tc.tile_pool(name="ln_tmp", bufs=2) as lnt,
                    tc.tile_pool(name="rmr", bufs=1) as rmrp,
                    tc.tile_pool(name="pv_ps", bufs=2, space="PSUM") as pv_ps,
                    tc.tile_pool(name="sc_ps", bufs=2, space="PSUM") as sc_ps,
                    tc.tile_pool(name="den_ps2", bufs=2,
                                 space="PSUM") as den_ps2,
                    tc.tile_pool(name="fin_ps", bufs=1, space="PSUM") as fin_ps,
                    tc.tile_pool(name="fin_sb", bufs=2) as fin_sb,
                    tc.tile_pool(name="es", bufs=5) as esp,
                    tc.tile_pool(name="rden", bufs=2) as rdenp,
                ):
                    r_q = rmrp.tile([128, S], BF16)
                    mr_q = rmrp.tile([128, S], BF16)
                    r_k = rmrp.tile([128, S], BF16)
                    mr_k = rmrp.tile([128, S], BF16)
                    r_ky = rmrp.tile([128, YL], BF16)
                    mr_ky = rmrp.tile([128, YL], BF16)

                    st_sb = stat_sbp.tile([1, statlen], FP32)
                    nc.sync.dma_start(st_sb[0:1, 0:2 * S],
                                      stats_out[0:1, 0:2 * S])
                    nc.sync.dma_start(st_sb[0:1, 2 * S:4 * S],
                                      stats_out[0:1, 2 * S:4 * S])
                    nc.sync.dma_start(st_sb[0:1, 4 * S:statlen],
                                      stats_out[0:1, 4 * S:statlen])
                    st_fr = st_sb

                    def make_rm(stat_off, ntok, r_t, mr_t):
                        nq = min(512, ntok)
                        for g in range(ntok // nq):
                            s_sl = st_fr[0:1,
                                         stat_off + g * nq:
                                         stat_off + (g + 1) * nq]
                            q_sl = st_fr[0:1,
                                         stat_off + ntok + g * nq:
                                         stat_off + ntok + (g + 1) * nq]
                            rsum = sc_ps.tile([128, nq], FP32, tag="sc")
                            rsq = sc_ps.tile([128, nq], FP32, tag="sc")
                            nc.tensor.matmul(rsum[:], lhsT=ones1_f32[:],
                                             rhs=s_sl, start=True, stop=True)
                            nc.tensor.matmul(rsq[:], lhsT=ones1_f32[:],
                                             rhs=q_sl, start=True, stop=True)
                            mean = lnt.tile([128, nq], FP32, tag="ln_mean", bufs=1)
                            nc.vector.tensor_scalar_mul(mean[:], rsum[:],
                                                        1.0 / D)
                            t_a = lnt.tile([128, nq], FP32, tag="ln_ta", bufs=1)
                            nc.vector.tensor_mul(t_a[:], mean[:], mean[:])
                            t_b = lnt.tile([128, nq], FP32, tag="ln_tb", bufs=1)
                            nc.vector.tensor_scalar_mul(t_b[:], rsq[:], 1.0 / D)
                            nc.vector.tensor_sub(t_b[:], t_b[:], t_a[:])
                            nc.scalar.activation(
                                t_a[:], t_b[:],
                                mybir.ActivationFunctionType.Sqrt,
                                bias=eps_t[:], scale=1.0)
                            nc.vector.reciprocal(
                                r_t[:, g * nq:(g + 1) * nq], t_a[:])
                            nc.vector.tensor_mul(
                                mr_t[:, g * nq:(g + 1) * nq], mean[:],
                                r_t[:, g * nq:(g + 1) * nq])

                    make_rm(0, S, r_q, mr_q)
                    make_rm(2 * S, S, r_k, mr_k)
                    make_rm(4 * S, YL, r_ky, mr_ky)

                    def ln_rope_head(jt, raws, r_t, mr_t, wnm, bnm, ntok,
                                     do_rope):
                        t1 = lnt.tile([128, ntok], BF16, tag="ln_t1", bufs=1)
                        nc.vector.tensor_mul(t1[:], raws[jt][:], r_t[:, :ntok])
                        nc.vector.tensor_sub(t1[:], t1[:], mr_t[:, :ntok])
                        nc.vector.tensor_scalar(
                            t1[:], t1[:], nrm[wnm][:, jt:jt + 1],
                            nrm[bnm][:, jt:jt + 1],
                            op0=mybir.AluOpType.mult, op1=mybir.AluOpType.add)
                        if not do_rope:
                            nc.vector.tensor_copy(raws[jt][:], t1[:])
                            return
                        hh = HD // 2
                        tsw = lnt.tile([128, ntok], BF16, tag="rope_sw", bufs=1)
                        nc.sync.dma_start(tsw[0:hh, :], t1[hh:HD, :])
                        nc.sync.dma_start(tsw[hh:HD, :], t1[0:hh, :])
                        p1 = lnt.tile([128, ntok], BF16, tag="rope_p1", bufs=1)
                        nc.vector.tensor_mul(p1[:], tsw[:], sin_sb[:, :ntok])
                        nc.vector.tensor_mul(raws[jt][:], t1[:],
                                             cos_sb[:, :ntok])
                        nc.vector.tensor_add(raws[jt][:], raws[jt][:], p1[:])

                    ones_bf128 = stat_sbp.tile([128, 128], BF16)
                    nc.gpsimd.memset(ones_bf128, 1.0)

                    wo_sb = []
                    for jc in range(NJ):
                        t = stat_sbp.tile([128, D], F32R, name=f"wo{jc}")
                        nc.sync.dma_start(
                            t[:], wo_d.ap()[jc * 128:(jc + 1) * 128, :])
                        wo_sb.append(t)

                    QW = 512
                    NQCH = S // QW

                    def attend_q(kns, vs, nkk, ht, q0, dst_op):
                        pv = pv_ps.tile([128, QW], FP32, tag="pv")
                        denr = den_ps2.tile([128, QW], FP32, tag="denr")
                        pend = []

                        def flush_one():
                            if pend:
                                pend.pop(0)()

                        for kkc in range(nkk):
                            sc = sc_ps.tile([128, QW], FP32, tag="sc")
                            nc.tensor.matmul(
                                sc[:], lhsT=kns[kkc],
                                rhs=qn_sb[ht][:, q0:q0 + QW],
                                start=True, stop=True)
                            es = esp.tile([128, QW], BF16, tag="es")
                            nc.scalar.activation(
                                es[:], sc[:],
                                mybir.ActivationFunctionType.Exp)
                            nc.tensor.matmul(
                                denr[:], lhsT=ones_bf128[:], rhs=es[:],
                                start=(kkc == 0), stop=(kkc == nkk - 1))

                            def mk_pv(kkc=kkc, es=es):
                                def run():
                                    nc.tensor.matmul(
                                        pv[:],
                                        lhsT=vs[kkc][:,
                                                     ht * 128:(ht + 1) * 128],
                                        rhs=es[:],
                                        start=(kkc == 0),
                                        stop=(kkc == nkk - 1))
                                return run
                            pend.append(mk_pv())
                            if len(pend) > 3:
                                flush_one()
                        while pend:
                            flush_one()
                        rden = rdenp.tile([128, QW], BF16, tag="rden")
                        nc.vector.reciprocal(rden[:], denr[:])
                        dst_op(pv, rden, q0)

                    NTC = QW // 128          # token tiles per q-chunk
                    NGF = D // 512           # N-groups in final proj
                    pending_fin = []
                    for qch in range(NQCH):
                        q0 = qch * QW
                        for ht in range(NJ):
                            if ht == 1 and pending_fin:
                                pending_fin.pop(0)()
                            if qch == 0:
                                # LN + rope just-in-time: head ht's attention
                                # starts as soon as its own LN is done.
                                ln_rope_head(ht, rawq, r_q, mr_q, "qn_w",
                                             "qn_b", S, True)
                                ln_rope_head(ht, rawk, r_k, mr_k, "kn_w",
                                             "kn_b", S, True)
                                ln_rope_head(ht, rawky, r_ky, mr_ky, "kyn_w",
                                             "kyn_b", YL, False)
                            kn_slices = [kn_sb[ht][:, c * 128:(c + 1) * 128]
                                         for c in range(NT)]

                            def self_out(pv, rden, q0, ht=ht):
                                nc.vector.tensor_mul(
                                    out_sb[ht][:, q0:q0 + QW], pv[:], rden[:])

                            attend_q(kn_slices, v_sb, NT, ht, q0, self_out)

                            kyn_slices = [kyn_sb[ht][:, c * 128:(c + 1) * 128]
                                          for c in range(NTY)]

                            def cross_out(pv, rden, q0, ht=ht):
                                nc.vector.tensor_scalar_mul(
                                    rden[:], rden[:], gate_b[:, ht:ht + 1])
                                tmp = rdenp.tile([128, QW], BF16,
                                                 tag="cross_tmp")
                                nc.vector.tensor_mul(tmp[:], pv[:], rden[:])
                                nc.vector.tensor_add(
                                    out_sb[ht][:, q0:q0 + QW],
                                    out_sb[ht][:, q0:q0 + QW], tmp[:])

                            attend_q(kyn_slices, vy_sb, NTY, ht, q0, cross_out)

                        # final projection + reduce-scatter, deferred
                        # by one q-chunk so attention matmuls hide its deps
                        def emit_fin(qch=qch):
                            NDH = 2 if NGF % 2 == 0 else 1
                            DH = D // NDH
                            for tt in range(qch * NTC, (qch + 1) * NTC):
                                for dh in range(NDH):
                                    fp = fin_ps.tile([128, DH], FP32,
                                                     tag="fin")
                                    NW = min(512, DH)
                                    for jc in range(NJ):
                                        for ng in range(DH // NW):
                                            c0 = dh * DH + ng * NW
                                            nc.tensor.matmul(
                                                fp[:, ng * NW:(ng + 1) * NW],
                                                lhsT=out_sb[jc][
                                                    :,
                                                    tt * 128:(tt + 1) * 128],
                                                rhs=wo_sb[jc][:, c0:c0 + NW],
                                                start=(jc == 0),
                                                stop=(jc == NJ - 1))
                                    fs = fin_sb.tile([128, DH], BF16,
                                                     tag="fstage")
                                    nc.vector.tensor_copy(fs[:], fp[:])
                                    nc.sync.dma_start(
                                        rs_in[tt * 128:(tt + 1) * 128,
                                              dh * DH:(dh + 1) * DH],
                                        fs[:])
                            nc.gpsimd.collective_compute(
                                "ReduceScatter", mybir.AluOpType.add,
                                replica_groups=groups,
                                ins=[rs_in[qch * QW:(qch + 1) * QW,
                                           :].opt()],
                                outs=[rs_out[qch * (QW // TP):
                                             (qch + 1) * (QW // TP),
                                             :].opt()],
                            )
                            nc.sync.dma_start(
                                out_d.ap()[qch * (QW // TP):
                                           (qch + 1) * (QW // TP), :],
                                rs_out[qch * (QW // TP):
                                       (qch + 1) * (QW // TP), :])
                        pending_fin.append(emit_fin)
                    while pending_fin:
                        pending_fin.pop(0)()
            outp_cm.__exit__(None, None, None)


# ======================= host side =========================================

def _perm_cols(cfg: Cfg, tp: int):
    """Global wq/wk/wky column indices for core tp: local heads with
    per-head even/odd interleave -> evens-first permutation."""
    cols = []
    for h in range(tp * cfg.HL, (tp + 1) * cfg.HL):
        base = h * cfg.HD
        cols.extend(range(base, base + cfg.HD, 2))
        cols.extend(range(base + 1, base + cfg.HD, 2))
    return np.asarray(cols)


def _nat_cols(cfg: Cfg, tp: int):
    return np.arange(tp * cfg.JL, (tp + 1) * cfg.JL)


def make_in_maps(cfg: Cfg, inputs: dict):
    bf = ml_dtypes.bfloat16
    sqhd = 1.0 / math.sqrt(cfg.HD)
    cos_t = np.asarray(inputs["freqs_cos"]).T.astype(np.float32)
    sin_t = np.asarray(inputs["freqs_sin"]).T.astype(np.float32)
    c2 = np.ascontiguousarray(np.vstack([cos_t, cos_t])).astype(bf)
    s2 = np.ascontiguousarray(np.vstack([-sin_t, sin_t])).astype(bf)
    inputs = {k: np.asarray(v) for k, v in inputs.items()}
    in_maps = []
    for c in range(cfg.TP * cfg.DP):
        dp, tp = divmod(c, cfg.TP)
        pc = _perm_cols(cfg, tp)
        ncol = _nat_cols(cfg, tp)

        def headcols(v, cols):
            return np.ascontiguousarray(
                v[cols].reshape(cfg.HL, cfg.HD).T).astype(np.float32)

        in_maps.append({
            "x": np.ascontiguousarray(inputs["x"][dp]).astype(np.float32),
            "y": np.ascontiguousarray(inputs["y"][dp]).astype(np.float32),
            "cos2": c2,
            "sin2": s2,
            "wq": np.ascontiguousarray(inputs["wq"][:, pc]).astype(bf),
            "wk": np.ascontiguousarray(inputs["wk"][:, pc]).astype(bf),
            "wv": np.ascontiguousarray(inputs["wv"][:, ncol]).astype(bf),
            "wky": np.ascontiguousarray(inputs["wk_y"][:, pc]).astype(bf),
            "wvy": np.ascontiguousarray(inputs["wv_y"][:, ncol]).astype(bf),
            "wo": np.ascontiguousarray(inputs["wo"][ncol, :]).astype(np.float32),
            # q-side norm params carry the 1/sqrt(HD) attention scale
            "qn_w": headcols(inputs["q_norm_w"] * sqhd, pc),
            "qn_b": headcols(inputs["q_norm_b"] * sqhd, pc),
            "kn_w": headcols(inputs["k_norm_w"], pc),
            "kn_b": headcols(inputs["k_norm_b"], pc),
            "kyn_w": headcols(inputs["ky_norm_w"], pc),
            "kyn_b": headcols(inputs["ky_norm_b"], pc),
            "gate": np.ascontiguousarray(
                inputs["gate"][tp * cfg.HL:(tp + 1) * cfg.HL][None, :]
            ).astype(np.float32),
        })
    return in_maps


def assemble(cfg: Cfg, results):
    B = cfg.DP
    out = np.empty((B, cfg.S, cfg.D), np.float32)
    NCH = cfg.S // 512          # one RS chunk per 512-token q-chunk
    chrows = 512
    shrows = chrows // cfg.TP
    for c in range(cfg.TP * cfg.DP):
        dp, tp = divmod(c, cfg.TP)
        sh = np.asarray(results[c]["out_shard"]).astype(np.float32)
        for ch in range(NCH):
            out[dp, ch * chrows + tp * shrows:
                ch * chrows + (tp + 1) * shrows, :] = \
                sh[ch * shrows:(ch + 1) * shrows]
    return out


_CACHE = {}


def build(cfg: Cfg):
    if cfg in _CACHE:
        return _CACHE[cfg]
    nc = bacc.Bacc("TRN2", target_bir_lowering=False, debug=False,
                   num_devices=cfg.TP * cfg.DP)
    emit(nc, cfg)
    nc.compile()
    _CACHE[cfg] = nc
    return nc


def kernel(**inputs) -> np.ndarray:
    cfg = Cfg()
    nc = build(cfg)
    in_maps = make_in_maps(cfg, inputs)
    res = bass_utils.run_bass_kernel_spmd(
        nc, in_maps, core_ids=list(range(cfg.TP * cfg.DP)))
    return assemble(cfg, res.results)
